# revision 27
# baseline (speedup 1.0000x reference)
"""Trainium2 Bass kernel for nn_DWMF_51874615001833 (sparse patch attention).

Computation (matches the reference nn.Module):
  - x[0:4]  : 4 local images [256, 80, 80], split into 64 patches of [256, 20, 20]
  - x[4]    : global image, bilinear-upsampled 2x and split into an 8x8 patch grid
  - per patch: head-averaged, query-averaged attention weights between the
    (local patch + sine pos-embed) queries and (global patch + pos-embed) keys
    produce a [20, 20] importance map; the local patch is scaled by
    0.6 * sigmoid(sal_w * imp + sal_b), patches are re-assembled, plus
    0.5 * upsampled-global.

Sharding: the 64 patch-attention instances are data-parallel across 8
NeuronCores (core d handles patches n = 8d..8d+7, which need only row d of the
global 8x8 key grid and produce a contiguous 40x80 block of the output).
Q/K projection weights are replicated. The host only slices inputs and
scatters the disjoint per-core output blocks.

Per-core device pipeline:
  - separable 2x bilinear upsample of the needed global-image slabs
  - qpT/kpT = W^T @ patch + (pos @ W^T + b)^T   (PE; pos/bias folded via an
    identity-weight matmul accumulated into the same PSUM)
  - per head: scores[l, s] = qp_h^T @ kp_h      (PE, K=32, row-group tiled)
  - exp on ACT (two heads per instruction, strided PSUM read)
  - row sums on DVE (tensor_scalar with accum_out, 2x perf mode)
  - colsum_s = sum_{h,l} exp[l,s] / rowsum[l]   (PE matvec accumulation)
  - importance = 0.6 / (1 + exp(-(a*colsum+b))) (ACT exp + DVE recip; a,b baked)
  - out = x_patch * importance (DVE) + 0.5 * up_global block (GPSIMD), DMA out
"""

import math

import numpy as np

import concourse.bass as bass
import concourse.tile as tile
from concourse import bacc, mybir
from concourse.bass_utils import run_bass_kernel_spmd
FP32 = mybir.dt.float32
R32 = mybir.dt.float32r
BF16 = mybir.dt.bfloat16


def _r(ap):
    return ap.bitcast(R32)
ALU = mybir.AluOpType
ACTF = mybir.ActivationFunctionType

D = 256
NHEADS = 8
PH = PW = 20
L = S = 400
LT = [(0, 128), (128, 128), (256, 128), (384, 16)]  # (start, size) tiles of L


# ----------------------------------------------------------------------------
# Host-side constant / input preparation
# ----------------------------------------------------------------------------

def _pos_embed_sine(h, w, F):
    scale = 2.0 * math.pi
    eps = 1e-6
    y = (np.arange(1, h + 1, dtype=np.float64) - 0.5) / (h + eps) * scale
    x = (np.arange(1, w + 1, dtype=np.float64) - 0.5) / (w + eps) * scale
    i = np.arange(F, dtype=np.float64)
    dim_t = 10000.0 ** (2.0 * np.floor(i / 2.0) / F)
    px = x[:, None] / dim_t
    py = y[:, None] / dim_t

    def interleave(p):
        return np.stack(
            [np.sin(p[:, 0::2]), np.cos(p[:, 1::2])], axis=-1
        ).reshape(p.shape[0], -1)

    px = interleave(px)
    py = interleave(py)
    pos_y = np.broadcast_to(py[:, None, :], (h, w, F))
    pos_x = np.broadcast_to(px[None, :, :], (h, w, F))
    return np.concatenate([pos_y, pos_x], axis=-1).transpose(2, 0, 1)  # [2F,h,w]


def _split_part(arr):
    """[256, ...] -> [128, 2, ...] (partition-inner, e-tile) device layout."""
    rest = arr.shape[1:]
    return np.ascontiguousarray(
        arr.reshape(2, 128, *rest).transpose(1, 0, *range(2, 2 + len(rest)))
    )


def prepare_inputs(x, in_proj_w, in_proj_b, sal_w, sal_b):
    """Returns (in_maps list of 8 dicts, a, b) ready for the device kernel."""
    x = np.ascontiguousarray(np.asarray(x, np.float32))
    in_proj_w = np.asarray(in_proj_w, np.float32)
    in_proj_b = np.asarray(in_proj_b, np.float32)
    inv = 1.0 / math.sqrt(D // NHEADS)

    pos = _pos_embed_sine(PH, PW, D // 2)            # [256, 20, 20] f64
    posf = pos.reshape(D, L).T                       # [400, 256] (l, e)
    Wq = in_proj_w[:D].astype(np.float64)
    Wk = in_proj_w[D:2 * D].astype(np.float64)
    bq = in_proj_b[:D].astype(np.float64)
    bk = in_proj_b[D:2 * D].astype(np.float64)

    wq_dev = _split_part((Wq.T * inv).astype(np.float32))          # [128,2,256]
    wk_dev = _split_part(Wk.T.astype(np.float32))                  # [128,2,256]
    bq_dev = _split_part(((posf @ Wq.T + bq) * inv).T.astype(np.float32))  # [128,2,400]
    bk_dev = _split_part((posf @ Wk.T + bk).T.astype(np.float32))          # [128,2,400]

    a = float(np.asarray(sal_w).reshape(-1)[0]) / (NHEADS * L)
    b = float(np.asarray(sal_b).reshape(-1)[0])

    glb = x[4]
    in_maps = []
    for d in range(8):
        img = d // 2
        nh0 = 2 * (d % 2)
        slab = x[img, :, nh0 * 20:(nh0 + 2) * 20, :]  # [256, 40, 80]
        xpat = slab.reshape(D, 2, 20, 4, 20).transpose(1, 3, 0, 2, 4).reshape(8, D, 400)
        rk = np.clip(np.arange(10 * d - 1, 10 * d + 11), 0, 79)
        ck = np.clip(np.arange(-1, 81), 0, 79)
        gkey = glb[:, rk][:, :, ck]                   # [256, 12, 82]
        gy0 = (d // 4) * 4 + 2 * (d % 2)
        gx0 = ((d // 2) % 2) * 4
        ro = np.clip(np.arange(10 * gy0 - 1, 10 * gy0 + 21), 0, 79)
        co = np.clip(np.arange(10 * gx0 - 1, 10 * gx0 + 41), 0, 79)
        gout = glb[:, ro][:, :, co]                   # [256, 22, 42]
        in_maps.append({
            "xpat": np.ascontiguousarray(
                xpat.reshape(8, 2, 128, 400).transpose(0, 2, 1, 3)),  # [8,128,2,400]
            "gkey": _split_part(gkey),                 # [128,2,12,82]
            "gout": _split_part(gout),                 # [128,2,22,42]
            "wq": wq_dev, "wk": wk_dev, "bq": bq_dev, "bk": bk_dev,
            "ident": np.eye(128, dtype=np.float32),
            "ones06": np.full((1, 128), 1.0, np.float32),
        })
    return in_maps, a, b


def gather_output(per_core_outs):
    """per_core_outs: list of 8 arrays [8, 128, 2, 400] -> [1, 256, 160, 160]."""
    out = np.empty((1, D, 160, 160), np.float32)
    for d in range(8):
        o = np.asarray(per_core_outs[d]).reshape(8, 128, 2, 400)
        o = o.transpose(0, 2, 1, 3).reshape(8, D, 20, 20)
        gy0 = (d // 4) * 4 + 2 * (d % 2)
        gx0 = ((d // 2) % 2) * 4
        for j in range(8):
            gy = gy0 + j // 4
            gx = gx0 + j % 4
            out[0, :, 20 * gy:20 * gy + 20, 20 * gx:20 * gx + 20] = o[j]
    return out


# ----------------------------------------------------------------------------
# Device kernel
# ----------------------------------------------------------------------------

def _upsample(nc, eng, pool, dst, g, R, Win, Rout, Wout, vscale, pfx="ups"):
    """Separable 2x bilinear upsample of pre-padded g [128,2,R,Win] -> dst
    [128,2,Rout,Wout]. vscale folds an extra constant into the vertical pass.
    Prescales run on DVE (2x tensor_scalar); the combines run on eng
    (gpsimd tensor_tensor) to spread load."""
    m = Wout // 2
    nr = Rout // 2
    t = pool.tile([128, 2, R, Win], FP32, tag=pfx + "_t")
    nc.vector.tensor_scalar_mul(t[:], g[:], 0.25)
    u = pool.tile([128, 2, R, Win], FP32, tag=pfx + "_u")
    nc.vector.tensor_scalar_mul(u[:], g[:], 0.75)
    hk = pool.tile([128, 2, R, Wout], FP32, tag=pfx + "_h")
    hv = hk.rearrange("p a r (m two) -> p a r m two", two=2)
    eng.tensor_tensor(out=hv[:, :, :, :, 0], in0=u[:, :, :, 1:1 + m],
                      in1=t[:, :, :, 0:m], op=ALU.add)
    eng.tensor_tensor(out=hv[:, :, :, :, 1], in0=u[:, :, :, 1:1 + m],
                      in1=t[:, :, :, 2:2 + m], op=ALU.add)
    t2 = pool.tile([128, 2, R, Wout], FP32, tag=pfx + "_t2")
    nc.vector.tensor_scalar_mul(t2[:], hk[:], 0.25 * vscale)
    u2 = pool.tile([128, 2, R, Wout], FP32, tag=pfx + "_u2")
    nc.vector.tensor_scalar_mul(u2[:], hk[:], 0.75 * vscale)
    dv = dst.rearrange("p a (r two) c -> p a r two c", two=2)
    eng.tensor_tensor(out=dv[:, :, :, 0, :], in0=u2[:, :, 1:1 + nr, :],
                      in1=t2[:, :, 0:nr, :], op=ALU.add)
    eng.tensor_tensor(out=dv[:, :, :, 1, :], in0=u2[:, :, 1:1 + nr, :],
                      in1=t2[:, :, 2:2 + nr, :], op=ALU.add)


def _body(nc, tc, pools, aps, a, b, ups_eng_name="gpsimd", out_add_eng="gpsimd"):
    const, ups, upc, xq_p, qk_p, e_p, r_p, i_p, o_p, ps, pc, pb = pools
    xpat, gkey, gout, wq, wk, bq, bk, identd, ones06d, outp = aps
    ups_eng = getattr(nc, ups_eng_name)
    oadd_eng = getattr(nc, out_add_eng)

    # constants
    wq_s = const.tile([128, 2, 256], R32, tag="wq")
    wk_s = const.tile([128, 2, 256], R32, tag="wk")
    bq_s = const.tile([128, 2, 400], R32, tag="bq")
    bk_s = const.tile([128, 2, 400], R32, tag="bk")
    nc.sync.dma_start(_r(wq_s[:]), _r(wq))
    nc.sync.dma_start(_r(wk_s[:]), _r(wk))
    nc.sync.dma_start(_r(bq_s[:]), _r(bq))
    nc.sync.dma_start(_r(bk_s[:]), _r(bk))
    ones06 = const.tile([1, 128], R32, tag="ones06")
    nc.sync.dma_start(ones06[:], ones06d)
    negb = const.tile([1, 1], FP32, tag="negb")
    nc.vector.memset(negb[:], -b)
    ident = const.tile([128, 128], R32, tag="ident")
    nc.sync.dma_start(_r(ident[:]), _r(identd))

    # upsampled global slabs. ukey is built in two column halves so the first
    # patches' key projections don't wait on the whole slab; the output slab
    # (needed only ~6 blocks in) is emitted lazily after the pipeline starts.
    gk_s = ups.tile([128, 2, 12, 82], FP32, tag="ups_g")
    nc.sync.dma_start(gk_s[:], gkey)
    ukeyA = const.tile([128, 2, 20, 80], R32, tag="ukeyA")
    ukeyB = const.tile([128, 2, 20, 80], R32, tag="ukeyB")
    _upsample(nc, nc.vector, ups, _r(ukeyA[:]), gk_s[:, :, :, 0:42], 12, 42, 20, 80, 1.0)

    def emit_late_upsample():
        _upsample(nc, nc.vector, ups, _r(ukeyB[:]), gk_s[:, :, :, 40:82], 12, 42, 20, 80, 1.0)
        go_s = ups.tile([128, 2, 22, 42], FP32, tag="ups_g")
        nc.sync.dma_start(go_s[:], gout)
        _upsample(nc, ups_eng, ups, uout, go_s, 22, 42, 40, 80, 0.5)

    uout = const.tile([128, 2, 40, 80], FP32, tag="uout")

    scr = const.tile([128, 400], BF16, tag="scr")  # write-only rowsum main out

    # Software-pipelined emission over 32 (patch, ltile) blocks: colsum blocks
    # trail the score/exp/rowsum blocks by LAG so the PE never waits on the
    # ACT->DVE softmax chain; per-patch outputs trail the patch's last colsum.
    LAG = 2
    P = {}  # per-patch state

    def emit_front(j):
        xq = xq_p.tile([128, 2, 400], R32, tag="xq")
        nc.sync.dma_start(_r(xq[:]), _r(xpat[j]))
        qp = qk_p.tile([128, 2, 400], R32, tag="qp")
        kp = qk_p.tile([128, 2, 400], R32, tag="kp")
        for (w_s, b_s, dst, is_q) in ((wq_s, bq_s, qp, True), (wk_s, bk_s, kp, False)):
            pt = ps.tile([128, 2, 512], FP32, tag="ps")
            for mt in range(2):
                for kt in range(2):
                    uk = ukeyA if j < 4 else ukeyB
                    rhs = xq[:, kt, :] if is_q else uk[:, kt, :, 20 * (j % 4):20 * (j % 4) + 20]
                    nc.tensor.matmul(
                        pt[:, mt, 0:400], lhsT=_r(w_s[:, kt, 128 * mt:128 * mt + 128]),
                        rhs=_r(rhs), start=(kt == 0), stop=False)
                nc.tensor.matmul(  # += I.T @ B  (bias + pos-embed fold)
                    pt[:, mt, 0:400], lhsT=_r(ident[:]), rhs=_r(b_s[:, mt, :]),
                    start=False, stop=True)
                nc.vector.tensor_copy(out=_r(dst[:, mt, :]), in_=pt[:, mt, 0:400])
        Cp = pc.tile([1, 400], FP32, tag="pc", name=f"Cp{j}")
        P[j] = dict(xq=xq, qp=qp, kp=kp, esbs={}, rinvs={}, Cp=Cp)

    def emit_block(j, lt):
        l0, lsz = LT[lt]
        qp, kp = P[j]["qp"], P[j]["kp"]
        st_tiles = []
        for g in range(4):  # head pairs (2g, 2g+1)
            st = ps.tile([128, 2, 512], FP32, tag="ps")
            for i in range(2):
                h = 2 * g + i
                ab = 32 * (h % 4)
                nc.tensor.matmul(
                    st[:lsz, i, 0:400], lhsT=_r(qp[ab:ab + 32, h // 4, l0:l0 + lsz]),
                    rhs=_r(kp[ab:ab + 32, h // 4, :]),
                    start=True, stop=True, tile_position=(ab, 0))
            st_tiles.append(st)
        rs = r_p.tile([128, 8], FP32, tag="rs")
        for g in range(4):
            et = e_p.tile([128, 2, 400], BF16, tag="e")
            nc.scalar.activation(
                out=et[:lsz, :, :], in_=st_tiles[g][:lsz, :, 0:400],
                func=ACTF.Exp)
            for i in range(2):
                P[j]["esbs"][(lt, 2 * g + i)] = et[:, i, :]
        for h in range(NHEADS):
            nc.vector.tensor_scalar(
                out=scr[:lsz, :], in0=P[j]["esbs"][(lt, h)][:lsz, :], scalar1=0.0,
                scalar2=0.0, op0=ALU.add, op1=ALU.add,
                accum_out=rs[:lsz, h:h + 1])
        ri = r_p.tile([128, 8], BF16, tag="ri")
        nc.vector.reciprocal(ri[:lsz, :], rs[:lsz, :])
        P[j]["rinvs"][lt] = ri

    def emit_colsums(j, lt):
        l0, lsz = LT[lt]
        for h in range(NHEADS):
            nc.tensor.matmul(
                P[j]["Cp"][0:1, :], lhsT=P[j]["rinvs"][lt][:lsz, h:h + 1],
                rhs=P[j]["esbs"][(lt, h)][:lsz, :],
                start=(lt == 0 and h == 0), stop=(lt == 3 and h == NHEADS - 1))

    def emit_back(j):
        # importance = 0.6 * sigmoid(a * C + b), via exp and reciprocal
        ev = i_p.tile([1, 400], R32, tag="ev")
        nc.scalar.activation(out=ev[:], in_=P[j]["Cp"][0:1, :], func=ACTF.Exp,
                             scale=-a, bias=negb[:])
        nc.vector.tensor_scalar_add(ev[:], ev[:], 1.0)
        nc.vector.reciprocal(_r(ev[:]), ev[:])
        ib_t = pb.tile([128, 400], FP32, tag="pb")
        ib = ib_t[:]
        nc.tensor.matmul(ib, lhsT=_r(ones06[0:1, :]), rhs=_r(ev[:]), start=True, stop=True)
        ot = o_p.tile([128, 2, 400], FP32, tag="ot")
        otv = ot.rearrange("p a (r c) -> p a r c", r=20)
        ry, cx = j // 4, j % 4
        xq = P[j]["xq"]
        for mt in range(2):
            nc.vector.scalar_tensor_tensor(
                out=ot[:, mt, :], in0=xq[:, mt, :].bitcast(FP32), scalar=0.6,
                in1=ib, op0=ALU.mult, op1=ALU.mult)
            oadd_eng.tensor_tensor(
                out=otv[:, mt], in0=otv[:, mt],
                in1=uout[:, mt, 20 * ry:20 * ry + 20, 20 * cx:20 * cx + 20],
                op=ALU.add)
        nc.sync.dma_start(outp[j], ot[:])
        del P[j]

    blocks = [(j, lt) for j in range(8) for lt in range(4)]
    n = len(blocks)
    for idx in range(n + LAG):
        if idx < n:
            j, lt = blocks[idx]
            if lt == 0:
                emit_front(j)
            emit_block(j, lt)
        if idx == 0:
            emit_late_upsample()
        if idx >= LAG:
            jc, ltc = blocks[idx - LAG]
            emit_colsums(jc, ltc)
            if ltc == 3:
                emit_back(jc)


def build(a, b, reps=1, ups_eng="gpsimd", out_add_eng="gpsimd"):
    nc = bacc.Bacc("TRN2", target_bir_lowering=False, debug=False, num_devices=8)
    xpat = nc.dram_tensor("xpat", (8, 128, 2, 400), R32, kind="ExternalInput").ap()
    gkey = nc.dram_tensor("gkey", (128, 2, 12, 82), FP32, kind="ExternalInput").ap()
    gout = nc.dram_tensor("gout", (128, 2, 22, 42), FP32, kind="ExternalInput").ap()
    wq = nc.dram_tensor("wq", (128, 2, 256), R32, kind="ExternalInput").ap()
    wk = nc.dram_tensor("wk", (128, 2, 256), R32, kind="ExternalInput").ap()
    bq = nc.dram_tensor("bq", (128, 2, 400), R32, kind="ExternalInput").ap()
    bk = nc.dram_tensor("bk", (128, 2, 400), R32, kind="ExternalInput").ap()
    identd = nc.dram_tensor("ident", (128, 128), R32, kind="ExternalInput").ap()
    ones06d = nc.dram_tensor("ones06", (1, 128), R32, kind="ExternalInput").ap()
    outp = nc.dram_tensor("out", (8, 128, 2, 400), FP32, kind="ExternalOutput").ap()
    aps = (xpat, gkey, gout, wq, wk, bq, bk, identd, ones06d, outp)

    with tile.TileContext(nc) as tc:
        with (
            tc.tile_pool(name="const", bufs=1) as const,
            tc.tile_pool(name="ups", bufs=1) as ups,
            tc.tile_pool(name="upc", bufs=3) as upc,
            tc.tile_pool(name="xq", bufs=3) as xq_p,
            tc.tile_pool(name="qk", bufs=2) as qk_p,
            tc.tile_pool(name="e", bufs=10) as e_p,
            tc.tile_pool(name="r", bufs=6) as r_p,
            tc.tile_pool(name="i", bufs=2) as i_p,
            tc.tile_pool(name="o", bufs=2) as o_p,
            tc.tile_pool(name="ps", bufs=3, space="PSUM") as ps,
            tc.tile_pool(name="pc", bufs=1, space="PSUM") as pc,
            tc.tile_pool(name="pb", bufs=1, space="PSUM") as pb,
        ):
            pools = (const, ups, upc, xq_p, qk_p, e_p, r_p, i_p, o_p, ps, pc, pb)
            with nc.allow_low_precision(reason="fp32r matmul operands"):
                if reps == 1:
                    _body(nc, tc, pools, aps, a, b, ups_eng, out_add_eng)
                else:
                    with tc.For_i(0, reps, 1):
                        _body(nc, tc, pools, aps, a, b, ups_eng, out_add_eng)
    nc.compile()
    return nc


# ----------------------------------------------------------------------------
# Entry point
# ----------------------------------------------------------------------------

def kernel(**inputs) -> np.ndarray:
    in_maps, a, b = prepare_inputs(
        inputs["x"], inputs["in_proj_w"], inputs["in_proj_b"],
        inputs["sal_w"], inputs["sal_b"])
    nc = build(a, b, reps=1)
    res = run_bass_kernel_spmd(nc, in_maps, core_ids=list(range(8)))
    return gather_output([r["out"] for r in res.results])


if __name__ == "__main__":
    rng = np.random.default_rng(0)
    ins = {
        "x": rng.standard_normal((5, 256, 80, 80), dtype=np.float32),
        "in_proj_w": (rng.standard_normal((768, 256)) * 0.05).astype(np.float32),
        "in_proj_b": (rng.standard_normal(768) * 0.05).astype(np.float32),
        "sal_w": rng.standard_normal(1).astype(np.float32),
        "sal_b": rng.standard_normal(1).astype(np.float32),
    }
    out = kernel(**ins)
    print("kernel out:", out.shape, out.dtype, float(np.abs(out).mean()))
